# revision 1
# baseline (speedup 1.0000x reference)
"""SuperposedExpert (K TT-factorized FFN paths + holographic routing) on 8 trn2 cores.

Strategy: expert x data parallel. Core c handles path k = c % 4 for token half
c // 4. On-device per core:
  1. logits/softmax gating from bf16 tokens (tiny matmuls on PE).
  2. TT expansion: W = G1 x_r G2 via rank-16 matmuls; the PSUM drain is stored
     through permuting DMAs that convert the Kronecker-mixed layout [(a,x),(b,y)]
     into the dense matmul layout [(a,b),(x,y)] -- W1 straight into SBUF
     (SBUF->SBUF scatter), W2 into DRAM, ring-fetched by the ffn2 loop.
  3. Dense bf16 FFN: hT = gelu(W1^T @ xT), oT = W2^T @ hT (PSUM fp32 accum).
  4. Scale by gate[n] and (1 + path_weight[d]), ReduceScatter(add) over the
     4 cores sharing the token half.
Host only reshapes/casts inputs and concatenates/transposes the output pieces.
"""

import numpy as np
import ml_dtypes

import concourse.bass as bass
import concourse.tile as tile
from concourse import bacc, mybir
from concourse.bass import ds, ts
from concourse.bass_utils import run_bass_kernel_spmd

BF16 = mybir.dt.bfloat16
F32 = mybir.dt.float32
AF = mybir.ActivationFunctionType

K = 4
D = 1024            # d_model
DFF = 4096          # d_ff
R = 16              # tt rank
IN1, IN2 = 32, 32   # d_model = IN1 * IN2   (a, b)
F1, F2 = 64, 64     # d_ff    = F1 * F2     (x, y)
NTOK = 2048
NCORES = 8
NHALF = NTOK // 2   # tokens per core
NCH = 512           # n-chunk (psum bank = 512 fp32)
NNCH = NHALF // NCH
GROUPS = [[0, 1, 2, 3], [4, 5, 6, 7]]


def _emit(nc, tc):
    # ---------------- I/O ----------------
    xT = nc.dram_tensor("xT", [D, NHALF], BF16, kind="ExternalInput")
    # TT cores packed for 4-way row-tiled rank-16 matmuls: row group i
    # (partitions 32i..32i+15) holds stationary chunk 4q+i / a replica of
    # the moving operand.
    g1pk = nc.dram_tensor("g1pk", [128, 512], BF16, kind="ExternalInput")
    g2pk = nc.dram_tensor("g2pk", [128, 2048], BF16, kind="ExternalInput")
    c1pk = nc.dram_tensor("c1pk", [128, 512], BF16, kind="ExternalInput")
    c2pk = nc.dram_tensor("c2pk", [128, 2048], BF16, kind="ExternalInput")
    pbT = nc.dram_tensor("pbT", [D, K], BF16, kind="ExternalInput")
    pw = nc.dram_tensor("pw", [128, 8], F32, kind="ExternalInput")
    sel = nc.dram_tensor("sel", [K, 1], F32, kind="ExternalInput")
    ones4 = nc.dram_tensor("ones4", [K, 1], F32, kind="ExternalInput")
    ones1 = nc.dram_tensor("ones1", [1, 128], F32, kind="ExternalInput")
    opiece = nc.dram_tensor("opiece", [D // K, NHALF], F32, kind="ExternalOutput")

    # DRAM scratch: W1/W2 in permuted dense layouts
    raw1p = nc.dram_tensor("raw1p", [128, 8, DFF], BF16)   # [(ah,al2,b), s, (x,y)]
    raw2p = nc.dram_tensor("raw2p", [128, 32, D], BF16)    # [(fh,f2), kc, (i1,i2)]
    cc_in = [nc.dram_tensor(f"cc_in{i}", [D, NCH], F32) for i in range(NNCH)]
    cc_out = [nc.dram_tensor(f"cc_out{i}", [D // K, NCH], F32) for i in range(NNCH)]

    with (
        tc.tile_pool(name="big", bufs=1) as big,
        tc.tile_pool(name="small", bufs=1) as small,
        tc.tile_pool(name="bounce", bufs=3) as bounce,
        tc.tile_pool(name="w2r", bufs=6) as w2r,
        tc.tile_pool(name="htp", bufs=1) as htp,
        tc.tile_pool(name="pp", bufs=8, space="PSUM") as pp,
    ):
        # ---------------- loads ----------------
        xt_sb = big.tile([128, 8, NHALF], BF16, tag="xt")
        nc.sync.dma_start(xt_sb, xT.ap().rearrange("(t p) n -> p t n", p=128))
        # W1 dense-layout resident: [p=(ah,al2,b), s=d-chunk, f=(x,y)]
        wb1 = big.tile([128, 8, DFF], BF16, tag="wb1")

        pbt_sb = small.tile([128, 8, K], BF16, tag="pbt")
        nc.sync.dma_start(pbt_sb, pbT.ap().rearrange("(t p) k -> p t k", p=128))

        pw_sb = small.tile([128, 8], F32, tag="pw")
        nc.sync.dma_start(pw_sb, pw.ap())
        sel_sb = small.tile([K, 1], F32, tag="sel")
        nc.sync.dma_start(sel_sb, sel.ap())
        ones4_sb = small.tile([K, 1], F32, tag="ones4")
        nc.sync.dma_start(ones4_sb, ones4.ap())
        ones1_sb = small.tile([1, 128], F32, tag="ones1")
        nc.sync.dma_start(ones1_sb, ones1.ap())

        g1_sb = small.tile([128, 512], BF16, tag="g1")
        nc.sync.dma_start(g1_sb, g1pk.ap())
        g2_sb = small.tile([128, 2048], BF16, tag="g2")
        nc.sync.dma_start(g2_sb, g2pk.ap())
        c1_sb = small.tile([128, 512], BF16, tag="c1")
        nc.sync.dma_start(c1_sb, c1pk.ap())
        c2_sb = small.tile([128, 2048], BF16, tag="c2")
        nc.sync.dma_start(c2_sb, c2pk.ap())

        # ---------------- gating ----------------
        # logits^T [K, n] = pbT^T @ xT, bf16 with fp32 accum; exp -> softmax
        expl = small.tile([K, NHALF], F32, tag="expl")
        for n2 in range(NNCH):
            lps = pp.tile([K, NCH], F32, tag="ps")
            for kc in range(8):
                nc.tensor.matmul(
                    lps, pbt_sb[:, kc], xt_sb[:, kc, ts(n2, NCH)],
                    start=(kc == 0), stop=(kc == 7),
                )
            nc.scalar.activation(expl[:, ts(n2, NCH)], lps, AF.Exp)

        gk = small.tile([1, NHALF], F32, tag="gk")
        rden = small.tile([1, NHALF], F32, tag="rden")
        for n2 in range(NNCH):
            den = pp.tile([1, NCH], F32, tag="ps")
            num = pp.tile([1, NCH], F32, tag="ps")
            nc.tensor.matmul(den, ones4_sb, expl[:, ts(n2, NCH)])
            nc.tensor.matmul(num, sel_sb, expl[:, ts(n2, NCH)])
            nc.vector.reciprocal(rden[:, ts(n2, NCH)], den)
            nc.vector.tensor_mul(gk[:, ts(n2, NCH)], num, rden[:, ts(n2, NCH)])

        # broadcast gate row to 128 partitions: gbc = ones1^T @ gk
        gbc_sb = small.tile([128, NHALF], F32, tag="gbc")
        for n2 in range(NNCH):
            gps = pp.tile([128, NCH], F32, tag="ps")
            nc.tensor.matmul(gps, ones1_sb, gk[:, ts(n2, NCH)])
            nc.vector.tensor_copy(gbc_sb[:, ts(n2, NCH)], gps)

        # ------------- TT expansion: rank-16 matmuls + permuting drains -------
        # raw1p viewed for stores: dims (ah, al2, s, x, b, y)
        raw1p_st = raw1p.ap().rearrange(
            "(ah al2 b) s (x y) -> ah al2 s x b y", ah=2, al2=2, x=64
        )
        for q in range(4):
            bts = [
                bounce.tile([128, 2048], BF16, tag="bt", name=f"bt1_{q}_{i}")
                for i in range(4)
            ]
            for nq in range(4):
                for i in range(4):
                    eps = pp.tile([128, NCH], F32, tag="ps", name=f"pe1_{q}_{nq}_{i}")
                    nc.tensor.matmul(
                        eps, g1_sb[ds(32 * i, R), ts(q, 128)],
                        g2_sb[ds(32 * i, R), ts(nq, NCH)],
                        tile_position=(32 * i, 0),
                    )
                    if (nq + i) % 2 == 0:
                        nc.vector.tensor_copy(bts[i][:, ts(nq, NCH)], eps)
                    else:
                        nc.scalar.activation(bts[i][:, ts(nq, NCH)], eps, AF.Copy)
            for i in range(4):
                mt = 4 * q + i
                # rows of chunk mt: a in {2mt, 2mt+1}; store per a2
                sv, ahv = mt // 2, mt % 2
                for a2 in range(2):
                    src = bts[i][ds(a2 * 64, 64), :].rearrange(
                        "x (b y) -> x b y", y=64
                    )
                    dst = raw1p_st[ds(ahv, 1), ds(a2, 1), ds(sv, 1)].squeeze()
                    nc.scalar.dma_start(dst, src)
        # load the dense-layout W1 back, one d-chunk at a time (pipelines
        # behind the stores on the other queue; ffn1 consumes s-ordered)
        for s in range(8):
            nc.sync.dma_start(wb1[:, s], raw1p[:, s, :])

        # W2: same, but store to DRAM raw2p [(fh f2), kc, (i1 i2)]
        raw2p_st = raw2p.ap().rearrange(
            "(fh f2) kc (i1 i2) -> fh kc i1 f2 i2", fh=2, i1=32
        )
        for q in range(4):
            bts2 = [
                bounce.tile([128, 2048], BF16, tag="bt", name=f"bt2_{q}_{i}")
                for i in range(4)
            ]
            for nq in range(4):
                for i in range(4):
                    eps = pp.tile([128, NCH], F32, tag="ps", name=f"pe2_{q}_{nq}_{i}")
                    nc.tensor.matmul(
                        eps, c1_sb[ds(32 * i, R), ts(q, 128)],
                        c2_sb[ds(32 * i, R), ts(nq, NCH)],
                        tile_position=(32 * i, 0),
                    )
                    if (nq + i) % 2 == 0:
                        nc.vector.tensor_copy(bts2[i][:, ts(nq, NCH)], eps)
                    else:
                        nc.scalar.activation(bts2[i][:, ts(nq, NCH)], eps, AF.Copy)
            for i in range(4):
                mt = 4 * q + i
                # rows of chunk mt: f1 in {4mt .. 4mt+3}
                for fl in range(4):
                    f1 = 4 * mt + fl
                    kcv, fhv = f1 // 2, f1 % 2
                    src = bts2[i][ds(fl * 32, 32), :].rearrange(
                        "i1 (f2 i2) -> i1 f2 i2", i2=32
                    )
                    dst = raw2p_st[ds(fhv, 1), ds(kcv, 1)].squeeze()
                    nc.scalar.dma_start(dst, src)

        # ---------------- main FFN, n-chunk at a time ----------------
        for nch in range(NNCH):
            ht = htp.tile([128, 32, NCH], BF16, tag="ht", name=f"ht_{nch}")
            # ffn1: hT[f, n] = gelu(sum_d W1[d, f] xT[d, n]); s-outer so the
            # first matmuls only need the first W1 d-chunk load
            for grp in range(4):
                ps_l1 = [
                    pp.tile([128, NCH], F32, tag="ps", name=f"ps1_{nch}_{grp}_{i}")
                    for i in range(8)
                ]
                for s in range(8):
                    for j in range(8):
                        m = grp * 8 + j
                        nc.tensor.matmul(
                            ps_l1[j], wb1[:, s, ts(m, 128)],
                            xt_sb[:, s, ts(nch, NCH)],
                            start=(s == 0), stop=(s == 7),
                        )
                for j in range(8):
                    nc.scalar.activation(
                        ht[:, grp * 8 + j], ps_l1[j], AF.Gelu_apprx_tanh
                    )

            # ffn2: oT[d, n] = sum_f W2[f, d] hT[f, n]; kc-outer, 8 live psum
            ps_l = [
                pp.tile([128, NCH], F32, tag="ps", name=f"ps2_{nch}_{i}")
                for i in range(8)
            ]
            for kc in range(32):
                wb2c = w2r.tile([128, D], BF16, tag="wb2c", name=f"w2_{nch}_{kc}")
                eng = nc.scalar if kc % 2 == 0 else nc.sync
                eng.dma_start(wb2c, raw2p[:, kc, :])
                for m2 in range(8):
                    nc.tensor.matmul(
                        ps_l[m2], wb2c[:, ts(m2, 128)], ht[:, kc],
                        start=(kc == 0), stop=(kc == 31),
                    )
            for m2 in range(8):
                ob = bounce.tile([128, NCH], F32, tag="ob", name=f"ob_{nch}_{m2}")
                nc.vector.tensor_mul(ob, ps_l[m2], gbc_sb[:, ts(nch, NCH)])
                nc.vector.tensor_scalar_mul(ob, ob, pw_sb[:, ds(m2, 1)])
                nc.sync.dma_start(cc_in[nch][ts(m2, 128), :], ob)

            # combine paths for this n-chunk (overlaps next chunk's compute)
            nc.gpsimd.collective_compute(
                "ReduceScatter",
                mybir.AluOpType.add,
                replica_groups=GROUPS,
                ins=[cc_in[nch][:]],
                outs=[cc_out[nch][:]],
            )
            nc.sync.dma_start(opiece[:, ts(nch, NCH)], cc_out[nch][:])


def build(verbose=False):
    nc = bacc.Bacc("TRN2", target_bir_lowering=False, debug=False, num_devices=NCORES)
    with tile.TileContext(nc) as tc:
        _emit(nc, tc)
    nc.compile()
    return nc


def make_in_maps(inputs):
    tokens = inputs["tokens"]
    bf = ml_dtypes.bfloat16
    in_maps = []
    for c in range(NCORES):
        half, k = c // 4, c % 4
        xt = np.ascontiguousarray(
            tokens[half * NHALF:(half + 1) * NHALF].T
        ).astype(bf)
        g1t = inputs["ffn1_core1"][k].transpose(2, 0, 1).reshape(R, IN1 * F1)
        g2 = inputs["ffn1_core2"][k].reshape(R, IN2 * F2)
        c1t = inputs["ffn2_core1"][k].transpose(2, 0, 1).reshape(R, F1 * IN1)
        c2 = inputs["ffn2_core2"][k].reshape(R, F2 * IN2)

        def pack_lhs(m):  # [R, 2048] -> [128, 512]: row group i gets chunk 4q+i
            out = np.zeros((128, 512), np.float32)
            for q in range(4):
                for i in range(4):
                    out[32 * i:32 * i + R, 128 * q:128 * (q + 1)] = \
                        m[:, 128 * (4 * q + i):128 * (4 * q + i + 1)]
            return out

        def pack_rhs(m):  # [R, 2048] -> [128, 2048]: replicate per row group
            out = np.zeros((128, 2048), np.float32)
            for i in range(4):
                out[32 * i:32 * i + R] = m
            return out
        pbt = np.ascontiguousarray(inputs["path_bases"].T).astype(bf)
        pwk = np.ascontiguousarray(
            (1.0 + inputs["path_weights"][k]).reshape(8, 128).T
        ).astype(np.float32)
        selk = np.zeros((K, 1), np.float32)
        selk[k, 0] = 1.0
        in_maps.append({
            "xT": xt,
            "g1pk": pack_lhs(g1t).astype(bf), "g2pk": pack_rhs(g2).astype(bf),
            "c1pk": pack_lhs(c1t).astype(bf), "c2pk": pack_rhs(c2).astype(bf),
            "pbT": pbt, "pw": pwk, "sel": selk,
            "ones4": np.ones((K, 1), np.float32),
            "ones1": np.ones((1, 128), np.float32),
        })
    return in_maps


def assemble(results):
    out = np.empty((NTOK, D), np.float32)
    for c in range(NCORES):
        half, k = c // 4, c % 4
        piece = results[c]["opiece"]  # [256 d-slice, 1024 tokens]
        out[half * NHALF:(half + 1) * NHALF, k * 256:(k + 1) * 256] = piece.T
    return out


_NC = None


def run(inputs, trace=False):
    global _NC
    if _NC is None:
        _NC = build()
    res = run_bass_kernel_spmd(
        _NC, make_in_maps(inputs), core_ids=list(range(NCORES)), trace=trace
    )
    return assemble(res.results), res


def kernel(**inputs):
    out, _ = run(inputs)
    return out



# revision 21
# speedup vs baseline: 1.0549x; 1.0549x over previous
"""SuperposedExpert (K TT-factorized FFN paths + holographic routing) on 8 trn2 cores.

Strategy: expert x data parallel. Core c handles path k = c % 4 for token half
c // 4. On-device per core:
  1. logits/softmax gating from bf16 tokens (tiny matmuls on PE), per n-chunk.
  2. TT expansion: W = G1 x_r G2 via rank-16 row-packed matmuls; PSUM drained
     (DVE/ACT split) to bounce tiles, then permuting SBUF->SBUF DMA scatters
     convert the Kronecker-mixed layout [(a,x),(b,y)] into the dense matmul
     layout [(a,b),(x,y)] directly into RESIDENT weight tiles (wb1, wb2).
     No DRAM roundtrip. W1 scatters issue on sync (HWDGE), W2 on gpsimd (SWDGE)
     to spread sequencer issue cost across idle engines.
  3. Dense bf16 FFN per 512-token chunk: hT = gelu(W1^T @ xT), oT = W2^T @ hT.
  4. Scale by gate[n] (partition-broadcast) and (1 + path_weight[d]), cast bf16,
     ReduceScatter(add) over the 4 cores sharing the token half, per chunk.
Host only reshapes/casts inputs and concatenates/transposes the output pieces.
"""

import numpy as np
import ml_dtypes

import concourse.bass as bass
import concourse.tile as tile
from concourse import bacc, mybir
from concourse.bass import ds, ts
from concourse.bass_utils import run_bass_kernel_spmd

BF16 = mybir.dt.bfloat16
F32 = mybir.dt.float32
AF = mybir.ActivationFunctionType

K = 4
D = 1024            # d_model
DFF = 4096          # d_ff
R = 16              # tt rank
IN1, IN2 = 32, 32   # d_model = IN1 * IN2   (a, b)
F1, F2 = 64, 64     # d_ff    = F1 * F2     (x, y)
NTOK = 2048
NCORES = 8
NHALF = NTOK // 2   # tokens per core
NCH = 512           # n-chunk (psum bank = 512 fp32)
NNCH = NHALF // NCH
GROUPS = [[0, 1, 2, 3], [4, 5, 6, 7]]


def _emit(nc, tc):
    # ---------------- I/O ----------------
    xT = nc.dram_tensor("xT", [D, NHALF], BF16, kind="ExternalInput")
    # TT cores packed for 4-way row-tiled rank-16 matmuls: row group i
    # (partitions 32i..32i+15) holds stationary chunk 4q+i / a replica of
    # the moving operand.
    g1pk = nc.dram_tensor("g1pk", [128, 512], BF16, kind="ExternalInput")
    g2pk = nc.dram_tensor("g2pk", [128, 2048], BF16, kind="ExternalInput")
    c1pk = nc.dram_tensor("c1pk", [128, 512], BF16, kind="ExternalInput")
    c2pk = nc.dram_tensor("c2pk", [128, 2048], BF16, kind="ExternalInput")
    pbT = nc.dram_tensor("pbT", [D, K], BF16, kind="ExternalInput")
    pw = nc.dram_tensor("pw", [128, 8], F32, kind="ExternalInput")
    sel = nc.dram_tensor("sel", [K, 1], F32, kind="ExternalInput")
    ones4 = nc.dram_tensor("ones4", [K, 1], F32, kind="ExternalInput")
    ones1 = nc.dram_tensor("ones1", [1, 128], F32, kind="ExternalInput")
    opiece = nc.dram_tensor("opiece", [D // K, NHALF], BF16, kind="ExternalOutput")

    # DRAM bounce buffers for the per-chunk ReduceScatter (bf16)
    cc_in = [nc.dram_tensor(f"cc_in{i}", [D, NCH], BF16) for i in range(NNCH)]
    cc_out = [nc.dram_tensor(f"cc_out{i}", [D // K, NCH], BF16) for i in range(NNCH)]
    # DRAM staging for the W permute: bounce tiles dumped verbatim (big
    # contiguous descriptors), permuting happens on the load side (DRAM src
    # APs are unrestricted; SBUF dst is partition-first = always legal).
    wdump = nc.dram_tensor("wdump", [32, 128, 2048], BF16)

    with (
        tc.tile_pool(name="big", bufs=1) as big,
        tc.tile_pool(name="small", bufs=1) as small,
        tc.tile_pool(name="xtp", bufs=1) as xtp,
        tc.tile_pool(name="gkp", bufs=2) as gkp,
        tc.tile_pool(name="bounce", bufs=2) as bounce,
        tc.tile_pool(name="obp", bufs=1) as obp,
        tc.tile_pool(name="pp", bufs=8, space="PSUM") as pp,
    ):
        # ---------------- resident weight tiles ----------------
        # W1 dense layout: [p=(a%4, b), s=d-chunk, f=(x,y)]
        wb1 = big.tile([128, 8, DFF], BF16, tag="wb1")
        # W2 dense layout: [p=(f1%2, f2), kc=f-chunk, d=(i1,i2)]
        wb2 = big.tile([128, 32, D], BF16, tag="wb2")
        ht = big.tile([128, 32, NCH], BF16, tag="ht")  # single buffer, serial reuse

        # ---------------- small loads ----------------
        pbt_sb = small.tile([128, 8, K], BF16, tag="pbt")
        nc.sync.dma_start(pbt_sb, pbT.ap().rearrange("(t p) k -> p t k", p=128))
        pw_sb = small.tile([128, 8], F32, tag="pw")
        nc.sync.dma_start(pw_sb, pw.ap())
        sel_sb = small.tile([K, 1], F32, tag="sel")
        nc.sync.dma_start(sel_sb, sel.ap())
        ones4_sb = small.tile([K, 1], F32, tag="ones4")
        nc.sync.dma_start(ones4_sb, ones4.ap())
        ones1_sb = small.tile([1, 128], F32, tag="ones1")
        nc.sync.dma_start(ones1_sb, ones1.ap())

        g1_sb = small.tile([128, 512], BF16, tag="g1")
        nc.sync.dma_start(g1_sb, g1pk.ap())
        g2_sb = small.tile([128, 2048], BF16, tag="g2")
        nc.sync.dma_start(g2_sb, g2pk.ap())
        c1_sb = small.tile([128, 512], BF16, tag="c1")
        nc.sync.dma_start(c1_sb, c1pk.ap())
        c2_sb = small.tile([128, 2048], BF16, tag="c2")
        nc.sync.dma_start(c2_sb, c2pk.ap())

        xt_view = xT.ap().rearrange("(t p) n -> p t n", p=128)
        xts = {}

        def load_xt(nch):
            xt_t = xtp.tile([128, 8, NCH], BF16, tag="xt", name=f"xt_{nch}")
            nc.sync.dma_start(xt_t, xt_view[:, :, ts(nch, NCH)])
            xts[nch] = xt_t

        load_xt(0)

        # ---------------- gating helper (per n-chunk) ----------------
        def gating(nch):
            xt_t = xts[nch]
            expl = gkp.tile([K, NCH], F32, tag="expl", name=f"expl_{nch}")
            lps = pp.tile([K, NCH], F32, tag="ps", name=f"lps_{nch}")
            for t in range(8):
                nc.tensor.matmul(
                    lps, pbt_sb[:, t], xt_t[:, t], start=(t == 0), stop=(t == 7)
                )
            nc.scalar.activation(expl, lps, AF.Exp)
            den = pp.tile([1, NCH], F32, tag="ps", name=f"den_{nch}")
            num = pp.tile([1, NCH], F32, tag="ps", name=f"num_{nch}")
            nc.tensor.matmul(den, ones4_sb, expl)
            nc.tensor.matmul(num, sel_sb, expl)
            gk = gkp.tile([1, NCH], F32, tag="gk", name=f"gk_{nch}")
            nc.vector.reciprocal(gk, den)
            nc.vector.tensor_mul(gk, num, gk)
            # broadcast gate row to 128 partitions via PE (DVE can't
            # partition-broadcast): gbc = ones1^T @ gk
            gbc = gkp.tile([128, NCH], BF16, tag="gbc", name=f"gbc_{nch}")
            gps = pp.tile([128, NCH], F32, tag="ps", name=f"gps_{nch}")
            nc.tensor.matmul(gps, ones1_sb, gk)
            nc.vector.tensor_copy(gbc, gps)
            return gbc

        gk0 = gating(0)

        # ------------- TT expansion W1: matmuls + drains + scatters -------
        # i-outer: each bounce tile is filled (4 matmuls + 4 drains), then
        # scattered and released before the next starts -> bufs=2 suffices.
        for q in range(4):
            for i in range(4):
                bt = bounce.tile([128, 2048], BF16, tag="bt", name=f"bt1_{q}_{i}")
                for nq in range(4):
                    eps = pp.tile([128, NCH], F32, tag="ps", name=f"pe1_{q}_{nq}_{i}")
                    nc.tensor.matmul(
                        eps, g1_sb[ds(32 * i, R), ts(q, 128)],
                        g2_sb[ds(32 * i, R), ts(nq, NCH)],
                        tile_position=(32 * i, 0),
                    )
                    if nq % 2 == 0:
                        nc.vector.tensor_copy(bt[:, ts(nq, NCH)], eps)
                    else:
                        nc.scalar.activation(bt[:, ts(nq, NCH)], eps, AF.Copy)
                mt = 4 * q + i
                nc.sync.dma_start(wdump[ds(mt, 1)].squeeze(), bt)
                if mt % 2 == 1:
                    # s-chunk sv complete: gather-load into wb1 (permuting)
                    sv = mt // 2
                    for mtb in (mt - 1, mt):
                        for a2 in range(2):
                            src = wdump[ds(mtb, 1)].squeeze().rearrange(
                                "(a2 x) (b y) -> a2 b x y", a2=2, y=64
                            )[ds(a2, 1)].squeeze()
                            dst = wb1[ds(64 * (mtb % 2) + 32 * a2, 32), sv] \
                                .rearrange("b (x y) -> b x y", y=64)
                            eng = nc.scalar if (mtb + a2) % 2 == 0 else nc.sync
                            eng.dma_start(dst, src)

        # ---------------- ffn1 for chunk 0 ----------------
        def ffn1(nch):
            xt_t = xts[nch]
            for grp in range(4):
                ps_l1 = [
                    pp.tile([128, NCH], F32, tag="ps", name=f"ps1_{nch}_{grp}_{i}")
                    for i in range(8)
                ]
                for s in range(8):
                    for j in range(8):
                        m = grp * 8 + j
                        nc.tensor.matmul(
                            ps_l1[j], wb1[:, s, ts(m, 128)], xt_t[:, s],
                            start=(s == 0), stop=(s == 7),
                        )
                for j in range(8):
                    nc.scalar.activation(
                        ht[:, grp * 8 + j], ps_l1[j], AF.Gelu_apprx_tanh
                    )

        ffn1(0)

        # ------------- TT expansion W2 (drains all-DVE, scatter on gpsimd) ---
        for q in range(4):
            for i in range(4):
                bt = bounce.tile([128, 2048], BF16, tag="bt", name=f"bt2_{q}_{i}")
                for nq in range(4):
                    eps = pp.tile([128, NCH], F32, tag="ps", name=f"pe2_{q}_{nq}_{i}")
                    nc.tensor.matmul(
                        eps, c1_sb[ds(32 * i, R), ts(q, 128)],
                        c2_sb[ds(32 * i, R), ts(nq, NCH)],
                        tile_position=(32 * i, 0),
                    )
                    nc.vector.tensor_copy(bt[:, ts(nq, NCH)], eps)
                mt = 4 * q + i
                nc.sync.dma_start(wdump[ds(16 + mt, 1)].squeeze(), bt)
                for fl in range(4):
                    f1 = 4 * mt + fl
                    kcv, fhv = f1 // 2, f1 % 2
                    src = wdump[ds(16 + mt, 1)].squeeze().rearrange(
                        "(fl i1) (f2 i2) -> fl f2 i1 i2", fl=4, i2=32
                    )[ds(fl, 1)].squeeze()
                    dst = wb2[ds(64 * fhv, 64), kcv].rearrange(
                        "f2 (i1 i2) -> f2 i1 i2", i2=32
                    )
                    eng = nc.scalar if fl % 2 == 0 else nc.sync
                    eng.dma_start(dst, src)

        # ---------------- ffn2 + scale + RS per chunk ----------------
        def ffn2(nch, gbc):
            ps_l = [
                pp.tile([128, NCH], F32, tag="ps", name=f"ps2_{nch}_{i}")
                for i in range(8)
            ]
            for kc in range(32):
                for m2 in range(8):
                    nc.tensor.matmul(
                        ps_l[m2], wb2[:, kc, ts(m2, 128)], ht[:, kc],
                        start=(kc == 0), stop=(kc == 31),
                    )
            ob = obp.tile([128, 8, NCH], BF16, tag="ob", name=f"ob_{nch}")
            for m2 in range(8):
                nc.vector.tensor_mul(ob[:, m2], ps_l[m2], gbc)
                nc.vector.tensor_scalar_mul(ob[:, m2], ob[:, m2], pw_sb[:, ds(m2, 1)])
            nc.sync.dma_start(
                cc_in[nch].ap().rearrange("(m2 p) n -> p m2 n", p=128), ob
            )
            nc.gpsimd.collective_compute(
                "ReduceScatter",
                mybir.AluOpType.add,
                replica_groups=GROUPS,
                ins=[cc_in[nch][:]],
                outs=[cc_out[nch][:]],
            )
            nc.sync.dma_start(opiece[:, ts(nch, NCH)], cc_out[nch][:])

        ffn2(0, gk0)

        for nch in range(1, NNCH):
            load_xt(nch)
            gbc = gating(nch)
            ffn1(nch)
            ffn2(nch, gbc)


def build(verbose=False):
    nc = bacc.Bacc("TRN2", target_bir_lowering=False, debug=False, num_devices=NCORES)
    with tile.TileContext(nc) as tc:
        _emit(nc, tc)
    nc.compile()
    return nc


def make_in_maps(inputs):
    tokens = inputs["tokens"]
    bf = ml_dtypes.bfloat16
    in_maps = []
    for c in range(NCORES):
        half, k = c // 4, c % 4
        xt = np.ascontiguousarray(
            tokens[half * NHALF:(half + 1) * NHALF].T
        ).astype(bf)
        g1t = inputs["ffn1_core1"][k].transpose(2, 0, 1).reshape(R, IN1 * F1)
        g2 = inputs["ffn1_core2"][k].reshape(R, IN2 * F2)
        c1t = inputs["ffn2_core1"][k].transpose(2, 0, 1).reshape(R, F1 * IN1)
        c2 = inputs["ffn2_core2"][k].reshape(R, F2 * IN2)

        def pack_lhs(m):  # [R, 2048] -> [128, 512]: row group i gets chunk 4q+i
            out = np.zeros((128, 512), np.float32)
            for q in range(4):
                for i in range(4):
                    out[32 * i:32 * i + R, 128 * q:128 * (q + 1)] = \
                        m[:, 128 * (4 * q + i):128 * (4 * q + i + 1)]
            return out

        def pack_rhs(m):  # [R, 2048] -> [128, 2048]: replicate per row group
            out = np.zeros((128, 2048), np.float32)
            for i in range(4):
                out[32 * i:32 * i + R] = m
            return out
        pbt = np.ascontiguousarray(inputs["path_bases"].T).astype(bf)
        pwk = np.ascontiguousarray(
            (1.0 + inputs["path_weights"][k]).reshape(8, 128).T
        ).astype(np.float32)
        selk = np.zeros((K, 1), np.float32)
        selk[k, 0] = 1.0
        in_maps.append({
            "xT": xt,
            "g1pk": pack_lhs(g1t).astype(bf), "g2pk": pack_rhs(g2).astype(bf),
            "c1pk": pack_lhs(c1t).astype(bf), "c2pk": pack_rhs(c2).astype(bf),
            "pbT": pbt, "pw": pwk, "sel": selk,
            "ones4": np.ones((K, 1), np.float32),
            "ones1": np.ones((1, 128), np.float32),
        })
    return in_maps


def assemble(results):
    out = np.empty((NTOK, D), np.float32)
    for c in range(NCORES):
        half, k = c // 4, c % 4
        piece = results[c]["opiece"]  # [256 d-slice, 1024 tokens] bf16
        out[half * NHALF:(half + 1) * NHALF, k * 256:(k + 1) * 256] = \
            piece.T.astype(np.float32)
    return out


_NC = None


def run(inputs, trace=False):
    global _NC
    if _NC is None:
        _NC = build()
    res = run_bass_kernel_spmd(
        _NC, make_in_maps(inputs), core_ids=list(range(NCORES)), trace=trace
    )
    return assemble(res.results), res


def kernel(**inputs):
    out, _ = run(inputs)
    return out


# revision 23
# speedup vs baseline: 1.1316x; 1.0727x over previous
"""SuperposedExpert (K TT-factorized FFN paths + holographic routing) on 8 trn2 cores.

Strategy: expert x data parallel. Core c handles path k = c % 4 for token half
c // 4. On-device per core:
  1. logits/softmax gating from bf16 tokens (tiny matmuls on PE), per n-chunk.
  2. TT expansion: W = G1 x_r G2 via rank-16 row-packed matmuls; PSUM drained
     (DVE/ACT split) to bounce tiles, then permuting SBUF->SBUF DMA scatters
     convert the Kronecker-mixed layout [(a,x),(b,y)] into the dense matmul
     layout [(a,b),(x,y)] directly into RESIDENT weight tiles (wb1, wb2).
     No DRAM roundtrip. W1 scatters issue on sync (HWDGE), W2 on gpsimd (SWDGE)
     to spread sequencer issue cost across idle engines.
  3. Dense bf16 FFN per 512-token chunk: hT = gelu(W1^T @ xT), oT = W2^T @ hT.
  4. Scale by gate[n] (partition-broadcast) and (1 + path_weight[d]), cast bf16,
     ReduceScatter(add) over the 4 cores sharing the token half, per chunk.
Host only reshapes/casts inputs and concatenates/transposes the output pieces.
"""

import numpy as np
import ml_dtypes

import concourse.bass as bass
import concourse.tile as tile
from concourse import bacc, mybir
from concourse.bass import ds, ts
from concourse.bass_utils import run_bass_kernel_spmd

BF16 = mybir.dt.bfloat16
F32 = mybir.dt.float32
AF = mybir.ActivationFunctionType

K = 4
D = 1024            # d_model
DFF = 4096          # d_ff
R = 16              # tt rank
IN1, IN2 = 32, 32   # d_model = IN1 * IN2   (a, b)
F1, F2 = 64, 64     # d_ff    = F1 * F2     (x, y)
NTOK = 2048
NCORES = 8
NHALF = NTOK // 2   # tokens per core
NCH = 512           # n-chunk (psum bank = 512 fp32)
NNCH = NHALF // NCH
GROUPS = [[0, 1, 2, 3], [4, 5, 6, 7]]


def _emit(nc, tc):
    # ---------------- I/O ----------------
    xT = nc.dram_tensor("xT", [D, NHALF], BF16, kind="ExternalInput")
    # TT cores packed for 4-way row-tiled rank-16 matmuls: row group i
    # (partitions 32i..32i+15) holds stationary chunk 4q+i / a replica of
    # the moving operand.
    g1pk = nc.dram_tensor("g1pk", [128, 512], BF16, kind="ExternalInput")
    g2pk = nc.dram_tensor("g2pk", [128, 2048], BF16, kind="ExternalInput")
    c1pk = nc.dram_tensor("c1pk", [128, 512], BF16, kind="ExternalInput")
    c2pk = nc.dram_tensor("c2pk", [128, 2048], BF16, kind="ExternalInput")
    pbT = nc.dram_tensor("pbT", [D, K], BF16, kind="ExternalInput")
    pw = nc.dram_tensor("pw", [128, 8], F32, kind="ExternalInput")
    sel = nc.dram_tensor("sel", [K, 1], F32, kind="ExternalInput")
    ones4 = nc.dram_tensor("ones4", [K, 1], F32, kind="ExternalInput")
    ones1 = nc.dram_tensor("ones1", [1, 128], F32, kind="ExternalInput")
    opiece = nc.dram_tensor("opiece", [D // K, NHALF], BF16, kind="ExternalOutput")

    # DRAM bounce buffers for the per-chunk ReduceScatter (bf16)
    cc_in = [nc.dram_tensor(f"cc_in{i}", [D, NCH], BF16) for i in range(NNCH)]
    cc_out = [nc.dram_tensor(f"cc_out{i}", [D // K, NCH], BF16) for i in range(NNCH)]
    # DRAM staging for the W permute: bounce tiles dumped verbatim (big
    # contiguous descriptors), permuting happens on the load side (DRAM src
    # APs are unrestricted; SBUF dst is partition-first = always legal).
    wdump = nc.dram_tensor("wdump", [32, 128, 2048], BF16)

    with (
        tc.tile_pool(name="big", bufs=1) as big,
        tc.tile_pool(name="small", bufs=1) as small,
        tc.tile_pool(name="xtp", bufs=1) as xtp,
        tc.tile_pool(name="gkp", bufs=2) as gkp,
        tc.tile_pool(name="bounce", bufs=2) as bounce,
        tc.tile_pool(name="obp", bufs=1) as obp,
        tc.tile_pool(name="pp", bufs=8, space="PSUM") as pp,
    ):
        # ---------------- resident weight tiles ----------------
        # W1 dense layout: [p=(a%4, b), s=d-chunk, f=(x,y)]
        wb1 = big.tile([128, 8, DFF], BF16, tag="wb1")
        # W2 dense layout: [p=(f1%2, f2), kc=f-chunk, d=(i1,i2)]
        wb2 = big.tile([128, 32, D], BF16, tag="wb2")
        ht = big.tile([128, 32, NCH], BF16, tag="ht")  # single buffer, serial reuse

        # ---------------- small loads ----------------
        pbt_sb = small.tile([128, 8, K], BF16, tag="pbt")
        nc.sync.dma_start(pbt_sb, pbT.ap().rearrange("(t p) k -> p t k", p=128))
        pw_sb = small.tile([128, 8], F32, tag="pw")
        nc.sync.dma_start(pw_sb, pw.ap())
        sel_sb = small.tile([K, 1], F32, tag="sel")
        nc.sync.dma_start(sel_sb, sel.ap())
        ones4_sb = small.tile([K, 1], F32, tag="ones4")
        nc.sync.dma_start(ones4_sb, ones4.ap())
        ones1_sb = small.tile([1, 128], F32, tag="ones1")
        nc.sync.dma_start(ones1_sb, ones1.ap())

        g1_sb = small.tile([128, 512], BF16, tag="g1")
        nc.sync.dma_start(g1_sb, g1pk.ap())
        g2_sb = small.tile([128, 2048], BF16, tag="g2")
        nc.sync.dma_start(g2_sb, g2pk.ap())
        c1_sb = small.tile([128, 512], BF16, tag="c1")
        nc.sync.dma_start(c1_sb, c1pk.ap())
        c2_sb = small.tile([128, 2048], BF16, tag="c2")
        nc.sync.dma_start(c2_sb, c2pk.ap())

        xt_view = xT.ap().rearrange("(t p) n -> p t n", p=128)
        xts = {}

        def load_xt(nch):
            xt_t = xtp.tile([128, 8, NCH], BF16, tag="xt", name=f"xt_{nch}")
            nc.sync.dma_start(xt_t, xt_view[:, :, ts(nch, NCH)])
            xts[nch] = xt_t

        load_xt(0)

        # ---------------- gating helper (per n-chunk) ----------------
        def gating(nch):
            xt_t = xts[nch]
            expl = gkp.tile([K, NCH], F32, tag="expl", name=f"expl_{nch}")
            lps = pp.tile([K, NCH], F32, tag="ps", name=f"lps_{nch}")
            for t in range(8):
                nc.tensor.matmul(
                    lps, pbt_sb[:, t], xt_t[:, t], start=(t == 0), stop=(t == 7)
                )
            nc.scalar.activation(expl, lps, AF.Exp)
            den = pp.tile([1, NCH], F32, tag="ps", name=f"den_{nch}")
            num = pp.tile([1, NCH], F32, tag="ps", name=f"num_{nch}")
            nc.tensor.matmul(den, ones4_sb, expl)
            nc.tensor.matmul(num, sel_sb, expl)
            gk = gkp.tile([1, NCH], F32, tag="gk", name=f"gk_{nch}")
            nc.vector.reciprocal(gk, den)
            nc.vector.tensor_mul(gk, num, gk)
            # broadcast gate row to 128 partitions via PE (DVE can't
            # partition-broadcast): gbc = ones1^T @ gk
            gbc = gkp.tile([128, NCH], BF16, tag="gbc", name=f"gbc_{nch}")
            gps = pp.tile([128, NCH], F32, tag="ps", name=f"gps_{nch}")
            nc.tensor.matmul(gps, ones1_sb, gk)
            nc.vector.tensor_copy(gbc, gps)
            return gbc

        gk0 = gating(0)

        # ------------- TT expansion W1: matmuls + drains + scatters -------
        # i-outer: each bounce tile is filled (4 matmuls + 4 drains), then
        # scattered and released before the next starts -> bufs=2 suffices.
        for q in range(4):
            for i in range(4):
                bt = bounce.tile([128, 2048], BF16, tag="bt", name=f"bt1_{q}_{i}")
                for nq in range(4):
                    eps = pp.tile([128, NCH], F32, tag="ps", name=f"pe1_{q}_{nq}_{i}")
                    nc.tensor.matmul(
                        eps, g1_sb[ds(32 * i, R), ts(q, 128)],
                        g2_sb[ds(32 * i, R), ts(nq, NCH)],
                        tile_position=(32 * i, 0),
                    )
                    if nq % 2 == 0:
                        nc.vector.tensor_copy(bt[:, ts(nq, NCH)], eps)
                    else:
                        nc.scalar.activation(bt[:, ts(nq, NCH)], eps, AF.Copy)
                mt = 4 * q + i
                nc.sync.dma_start(wdump[ds(mt, 1)].squeeze(), bt)
                if mt % 2 == 1:
                    # s-chunk sv complete: gather-load into wb1 (permuting)
                    sv = mt // 2
                    for mtb in (mt - 1, mt):
                        for a2 in range(2):
                            src = wdump[ds(mtb, 1)].squeeze().rearrange(
                                "(a2 x) (b y) -> a2 b x y", a2=2, y=64
                            )[ds(a2, 1)].squeeze()
                            dst = wb1[ds(64 * (mtb % 2) + 32 * a2, 32), sv] \
                                .rearrange("b (x y) -> b x y", y=64)
                            nc.gpsimd.dma_start(dst, src)

        # ---------------- ffn1 for chunk 0 ----------------
        def ffn1(nch):
            xt_t = xts[nch]
            for grp in range(4):
                ps_l1 = [
                    pp.tile([128, NCH], F32, tag="ps", name=f"ps1_{nch}_{grp}_{i}")
                    for i in range(8)
                ]
                for s in range(8):
                    for j in range(8):
                        m = grp * 8 + j
                        nc.tensor.matmul(
                            ps_l1[j], wb1[:, s, ts(m, 128)], xt_t[:, s],
                            start=(s == 0), stop=(s == 7),
                        )
                for j in range(8):
                    nc.scalar.activation(
                        ht[:, grp * 8 + j], ps_l1[j], AF.Gelu_apprx_tanh
                    )

        ffn1(0)

        # ------------- TT expansion W2 (drains all-DVE, scatter on gpsimd) ---
        for q in range(4):
            for i in range(4):
                bt = bounce.tile([128, 2048], BF16, tag="bt", name=f"bt2_{q}_{i}")
                for nq in range(4):
                    eps = pp.tile([128, NCH], F32, tag="ps", name=f"pe2_{q}_{nq}_{i}")
                    nc.tensor.matmul(
                        eps, c1_sb[ds(32 * i, R), ts(q, 128)],
                        c2_sb[ds(32 * i, R), ts(nq, NCH)],
                        tile_position=(32 * i, 0),
                    )
                    nc.vector.tensor_copy(bt[:, ts(nq, NCH)], eps)
                mt = 4 * q + i
                nc.sync.dma_start(wdump[ds(16 + mt, 1)].squeeze(), bt)
                for fl in range(4):
                    f1 = 4 * mt + fl
                    kcv, fhv = f1 // 2, f1 % 2
                    src = wdump[ds(16 + mt, 1)].squeeze().rearrange(
                        "(fl i1) (f2 i2) -> fl f2 i1 i2", fl=4, i2=32
                    )[ds(fl, 1)].squeeze()
                    dst = wb2[ds(64 * fhv, 64), kcv].rearrange(
                        "f2 (i1 i2) -> f2 i1 i2", i2=32
                    )
                    nc.gpsimd.dma_start(dst, src)

        # ---------------- ffn2 + scale + RS per chunk ----------------
        def ffn2(nch, gbc):
            ps_l = [
                pp.tile([128, NCH], F32, tag="ps", name=f"ps2_{nch}_{i}")
                for i in range(8)
            ]
            for kc in range(32):
                for m2 in range(8):
                    nc.tensor.matmul(
                        ps_l[m2], wb2[:, kc, ts(m2, 128)], ht[:, kc],
                        start=(kc == 0), stop=(kc == 31),
                    )
            ob = obp.tile([128, 8, NCH], BF16, tag="ob", name=f"ob_{nch}")
            for m2 in range(8):
                nc.vector.tensor_mul(ob[:, m2], ps_l[m2], gbc)
                nc.vector.tensor_scalar_mul(ob[:, m2], ob[:, m2], pw_sb[:, ds(m2, 1)])
            nc.sync.dma_start(
                cc_in[nch].ap().rearrange("(m2 p) n -> p m2 n", p=128), ob
            )
            nc.gpsimd.collective_compute(
                "ReduceScatter",
                mybir.AluOpType.add,
                replica_groups=GROUPS,
                ins=[cc_in[nch][:]],
                outs=[cc_out[nch][:]],
            )
            nc.sync.dma_start(opiece[:, ts(nch, NCH)], cc_out[nch][:])

        ffn2(0, gk0)

        for nch in range(1, NNCH):
            load_xt(nch)
            gbc = gating(nch)
            ffn1(nch)
            ffn2(nch, gbc)


def build(verbose=False):
    nc = bacc.Bacc("TRN2", target_bir_lowering=False, debug=False, num_devices=NCORES)
    with tile.TileContext(nc) as tc:
        _emit(nc, tc)
    nc.compile()
    return nc


def make_in_maps(inputs):
    tokens = inputs["tokens"]
    bf = ml_dtypes.bfloat16
    in_maps = []
    for c in range(NCORES):
        half, k = c // 4, c % 4
        xt = np.ascontiguousarray(
            tokens[half * NHALF:(half + 1) * NHALF].T
        ).astype(bf)
        g1t = inputs["ffn1_core1"][k].transpose(2, 0, 1).reshape(R, IN1 * F1)
        g2 = inputs["ffn1_core2"][k].reshape(R, IN2 * F2)
        c1t = inputs["ffn2_core1"][k].transpose(2, 0, 1).reshape(R, F1 * IN1)
        c2 = inputs["ffn2_core2"][k].reshape(R, F2 * IN2)

        def pack_lhs(m):  # [R, 2048] -> [128, 512]: row group i gets chunk 4q+i
            out = np.zeros((128, 512), np.float32)
            for q in range(4):
                for i in range(4):
                    out[32 * i:32 * i + R, 128 * q:128 * (q + 1)] = \
                        m[:, 128 * (4 * q + i):128 * (4 * q + i + 1)]
            return out

        def pack_rhs(m):  # [R, 2048] -> [128, 2048]: replicate per row group
            out = np.zeros((128, 2048), np.float32)
            for i in range(4):
                out[32 * i:32 * i + R] = m
            return out
        pbt = np.ascontiguousarray(inputs["path_bases"].T).astype(bf)
        pwk = np.ascontiguousarray(
            (1.0 + inputs["path_weights"][k]).reshape(8, 128).T
        ).astype(np.float32)
        selk = np.zeros((K, 1), np.float32)
        selk[k, 0] = 1.0
        in_maps.append({
            "xT": xt,
            "g1pk": pack_lhs(g1t).astype(bf), "g2pk": pack_rhs(g2).astype(bf),
            "c1pk": pack_lhs(c1t).astype(bf), "c2pk": pack_rhs(c2).astype(bf),
            "pbT": pbt, "pw": pwk, "sel": selk,
            "ones4": np.ones((K, 1), np.float32),
            "ones1": np.ones((1, 128), np.float32),
        })
    return in_maps


def assemble(results):
    out = np.empty((NTOK, D), np.float32)
    for c in range(NCORES):
        half, k = c // 4, c % 4
        piece = results[c]["opiece"]  # [256 d-slice, 1024 tokens] bf16
        out[half * NHALF:(half + 1) * NHALF, k * 256:(k + 1) * 256] = \
            piece.T.astype(np.float32)
    return out


_NC = None


def run(inputs, trace=False):
    global _NC
    if _NC is None:
        _NC = build()
    res = run_bass_kernel_spmd(
        _NC, make_in_maps(inputs), core_ids=list(range(NCORES)), trace=trace
    )
    return assemble(res.results), res


def kernel(**inputs):
    out, _ = run(inputs)
    return out
